# revision 29
# baseline (speedup 1.0000x reference)
import sys
import numpy as np

sys.path.insert(0, "/opt/trn_rl_repo")

import concourse.bass as bass  # noqa: E402
import concourse.mybir as mybir  # noqa: E402
import concourse.tile as tile  # noqa: E402
from concourse import bacc  # noqa: E402
from concourse.bass_utils import run_bass_kernel_spmd  # noqa: E402
from contextlib import ExitStack  # noqa: E402

F32 = mybir.dt.float32
F32R = mybir.dt.float32r
ACTF = mybir.ActivationFunctionType
ALU = mybir.AluOpType
AXX = mybir.AxisListType.X

NCORES = 8
B, D, H1, H2, KH = 16384, 512, 1024, 2048, 11
BC = B // NCORES            # 2048 rows per core
EPS = 1e-5
THR = 0.25
LN10 = float(np.log(10.0))
N_INV = 1.0 / float(B)

_BUILD_CACHE = {}
TRACE = False          # test harness hook: set True to capture a profile
LAST_RESULT = None     # test harness hook: BassKernelResults of last run


def _build():
    if "nc" in _BUILD_CACHE:
        return _BUILD_CACHE["nc"]
    nc = bacc.Bacc(None, target_bir_lowering=False, debug=False)

    # ------- external I/O (per core) -------
    xn_d = nc.dram_tensor("xn", [128, 16, 512], F32, kind="ExternalInput")
    xt_d = nc.dram_tensor("xt", [128, 4, 2048], F32, kind="ExternalInput")
    wc1_d = nc.dram_tensor("wc1", [128, 4, H1], F32, kind="ExternalInput")
    wc2_d = nc.dram_tensor("wc2", [128, 8, H2], F32, kind="ExternalInput")
    wc3_d = nc.dram_tensor("wc3", [128, 16, KH], F32, kind="ExternalInput")
    rw1_d = nc.dram_tensor("rw1", [KH, 128, 4, H1], F32, kind="ExternalInput")
    rw1m_d = nc.dram_tensor("rw1m", [2, 128, 4, H1], F32, kind="ExternalInput")
    rw2_d = nc.dram_tensor("rw2", [KH, 128, 8, 512], F32, kind="ExternalInput")
    rw3_d = nc.dram_tensor("rw3", [128, 4, KH], F32, kind="ExternalInput")
    vecl1_d = nc.dram_tensor("vecl1", [128, 96, 2], F32, kind="ExternalInput")
    vecl2_d = nc.dram_tensor("vecl2", [128, 60, 2], F32, kind="ExternalInput")
    row11_d = nc.dram_tensor("row11", [128, 4, KH], F32, kind="ExternalInput")
    eye11_d = nc.dram_tensor("eye11", [11, 11], F32, kind="ExternalInput")

    of_d = nc.dram_tensor("of", [128, 16], F32, kind="ExternalOutput")
    ol_d = nc.dram_tensor("ol", [128, 16, KH], F32, kind="ExternalOutput")
    op_d = nc.dram_tensor("op", [128, 16, KH], F32, kind="ExternalOutput")
    oa_d = nc.dram_tensor("oa", [128, 16], F32, kind="ExternalOutput")

    # ------- internal DRAM -------
    gin = nc.dram_tensor("gin", [128, 2052], F32, kind="Internal")
    gout = nc.dram_tensor("gout", [128, 2052], F32, kind="Internal",
                          addr_space="Shared")
    qg_in = nc.dram_tensor("qg_in", [128, 32], F32, kind="Internal")
    qg_out = nc.dram_tensor("qg_out", [8, 128, 32], F32, kind="Internal",
                            addr_space="Shared")
    sc_in = nc.dram_tensor("sc_in", [128, 32], F32, kind="Internal")
    sc_out = nc.dram_tensor("sc_out", [128, 32], F32, kind="Internal",
                            addr_space="Shared")
    sh_in = nc.dram_tensor("sh_in", [128, 88], F32, kind="Internal")
    sh_out = nc.dram_tensor("sh_out", [128, 88], F32, kind="Internal",
                            addr_space="Shared")
    h2s = nc.dram_tensor("h2s", [128, 16, 2048], F32, kind="Internal")
    # r2 preacts round-trip in bf16: only the (smooth) pcts path flows
    # through them, and halving this 92MB of traffic matters more than the
    # ~2^-9 rounding it adds there.
    r2s = nc.dram_tensor("r2s", [KH, 128, 4, 2048], mybir.dt.bfloat16,
                         kind="Internal")

    RG = [list(range(NCORES))]

    with tile.TileContext(nc) as tc, ExitStack() as top:
        const = top.enter_context(tc.tile_pool(name="const", bufs=1))
        # long-lived big tiles; strict LIFO pool discipline, so the pools
        # that close earliest open last.
        pxr = top.enter_context(tc.tile_pool(name="pxr", bufs=1))
        sA1 = top.enter_context(ExitStack())      # a1T: closed after C-L2
        a1pool = sA1.enter_context(tc.tile_pool(name="a1pool", bufs=1))
        sW = top.enter_context(ExitStack())       # wc1: closed after C-L1
        pcW = sW.enter_context(tc.tile_pool(name="pcW", bufs=1))
        sG = top.enter_context(ExitStack())       # G: closed after B
        gpool = sG.enter_context(tc.tile_pool(name="gpool", bufs=1))
        pa_top = top.enter_context(ExitStack())   # xn: closed after A
        pa = pa_top.enter_context(tc.tile_pool(name="pa", bufs=1))

        # x natural-layout shard loads first (per-tile chunks so the gram
        # matmuls pipeline with the loads)
        xn_sb = pa.tile([128, 16, 512], F32)
        for t in range(16):
            nc.sync.dma_start(xn_sb[:, t, :], xn_d[:, t, :])

        ones = const.tile([128, 1], F32)
        nc.vector.memset(ones, 1.0)
        epsT = const.tile([128, 1], F32)
        nc.vector.memset(epsT, EPS)
        row11_sb = const.tile([128, 4, KH], F32)
        nc.sync.dma_start(row11_sb[:], row11_d[:])
        vecl1_sb = const.tile([128, 96, 2], F32)
        nc.sync.dma_start(vecl1_sb[:], vecl1_d[:])
        vecl2_sb = const.tile([128, 60, 2], F32)
        nc.sync.dma_start(vecl2_sb[:], vecl2_d[:])
        eye11_sb = const.tile([11, 11], F32)
        nc.sync.dma_start(eye11_sb[:], eye11_d[:])
        stats_cls = const.tile([128, 16, 2], F32)
        stats_hd = const.tile([128, 44, 2], F32)
        a1c = const.tile([128, 8], F32)
        b1c = const.tile([128, 8], F32)
        A1H = const.tile([128, KH, 8], F32)
        B1H = const.tile([128, KH, 8], F32)
        a2c = const.tile([128, 16], F32)
        b2c = const.tile([128, 16], F32)
        A2H = const.tile([128, KH, 4], F32)
        B2H = const.tile([128, KH, 4], F32)
        p_sb = const.tile([128, 16, KH], F32)
        pcts_sb = const.tile([128, 16, KH], F32)
        logits_sb = const.tile([128, 16, KH], F32)
        final_sb = const.tile([128, 16], F32)
        atpm_sb = const.tile([128, 16], F32)

        xtr_sb = pxr.tile([128, 4, 2048], F32R)
        nc.sync.dma_start(xtr_sb[:], xt_d[:].bitcast(F32R))
        a1T = a1pool.tile([128, 8, 2048], F32)
        wc1_sb = pcW.tile([128, 4, H1], F32)
        nc.sync.dma_start(wc1_sb[:], wc1_d[:])
        G32g = gpool.tile([128, 4, 512], F32)
        Gr = gpool.tile([128, 4, 512], F32R)
        csg = gpool.tile([128, 4], F32)

        # ================= phase A: Gram + colsum =================
        with ExitStack() as sA:
            psA = sA.enter_context(tc.tile_pool(name="psA", bufs=1, space="PSUM"))
            G32 = pa.tile([128, 4, 512], F32)
            cs0 = pa.tile([128, 4], F32)
            psgs = [psA.tile([128, 512], F32, tag=f"g{m}", name=f"psg{m}")
                    for m in range(4)]
            pscs = [psA.tile([128, 1], F32, tag=f"c{m}", name=f"psc{m}")
                    for m in range(4)]
            for t in range(16):
                for m in range(4):
                    nc.tensor.matmul(psgs[m][:], xn_sb[:, t, m * 128:(m + 1) * 128],
                                     xn_sb[:, t, :], start=(t == 0), stop=(t == 15))
                    nc.tensor.matmul(pscs[m][:], xn_sb[:, t, m * 128:(m + 1) * 128],
                                     ones[:], start=(t == 0), stop=(t == 15))
            for m in range(4):
                nc.vector.tensor_copy(G32[:, m, :], psgs[m][:])
                nc.vector.tensor_copy(cs0[:, m:m + 1], pscs[m][:])
            nc.gpsimd.dma_start(gin[:, 0:2048], G32[:].rearrange("p m d -> p (m d)"))
            nc.gpsimd.dma_start(gin[:, 2048:2052], cs0[:])
        pa_top.close()

        nc.gpsimd.collective_compute(
            "AllReduce", ALU.add, replica_groups=RG,
            ins=[gin[:].opt()], outs=[gout[:].opt()])
        nc.sync.dma_start(G32g[:].rearrange("p m d -> p (m d)"), gout[:, 0:2048])
        nc.sync.dma_start(Gr[:].rearrange("p m d -> p (m d)"),
                          gout[:, 0:2048].bitcast(F32R))
        nc.sync.dma_start(csg[:], gout[:, 2048:2052])

        # stats helper: for a first-layer weight W [512, H] compute raw
        #   s_j = (W^T cs)_j            (sum of h over full batch)
        #   q_j = sum_d1 W.(G@W) [.,j]  (sum of h^2 over full batch)
        def emit_l1_raw_stats(W32, Wr, n_jt, qs_out, use_f32r, pools):
            tmpP, psP = pools
            Hn = n_jt * 128
            tmp = tmpP.tile([128, 4, Hn], F32, tag="tmp")
            for m4 in range(4):
                for nch in range(Hn // 512):
                    psgw = psP.tile([128, 512], F32, tag="gw")
                    for kt in range(4):
                        nc.tensor.matmul(
                            psgw[:],
                            (Gr if use_f32r else G32g)[:, kt, m4 * 128:(m4 + 1) * 128],
                            (Wr if use_f32r else W32)[:, kt, nch * 512:(nch + 1) * 512],
                            start=(kt == 0), stop=(kt == 3))
                    nc.vector.tensor_tensor(
                        tmp[:, m4, nch * 512:(nch + 1) * 512], psgw[:],
                        W32[:, m4, nch * 512:(nch + 1) * 512], ALU.mult)
            for jc in range(n_jt):
                psq = psP.tile([128, 1], F32, tag="q")
                for m4 in range(4):
                    nc.tensor.matmul(psq[:], tmp[:, m4, jc * 128:(jc + 1) * 128],
                                     ones[:], start=(m4 == 0), stop=(m4 == 3))
                pss = psP.tile([128, 1], F32, tag="s")
                for kt in range(4):
                    nc.tensor.matmul(pss[:], W32[:, kt, jc * 128:(jc + 1) * 128],
                                     csg[:, kt:kt + 1], start=(kt == 0), stop=(kt == 3))
                nc.vector.tensor_copy(qs_out[:, jc, 0:1], pss[:])
                nc.vector.tensor_copy(qs_out[:, jc, 1:2], psq[:])

        # raw (s, q) -> bn scale/shift: alpha = g*rsqrt(var+eps),
        # beta = be - mean*alpha.  All [128, ...] elementwise; sq[..., 0]=s,
        # sq[..., 1]=q; g_be[..., 0]=gamma, [..., 1]=beta_in.
        def emit_alpha_beta(s_ap, q_ap, g_ap, be_ap, alphaT, betaT, scr, shp, tg):
            mean = scr.tile(list(shp), F32, tag=f"mean{tg}")
            var = scr.tile(list(shp), F32, tag=f"var{tg}")
            m2t = scr.tile(list(shp), F32, tag=f"m2t{tg}")
            nc.vector.tensor_scalar_mul(mean[:], s_ap, N_INV)
            nc.vector.tensor_scalar_mul(var[:], q_ap, N_INV)
            nc.vector.tensor_tensor(m2t[:], mean[:], mean[:], ALU.mult)
            nc.vector.tensor_sub(var[:], var[:], m2t[:])
            nc.scalar.activation(var[:], var[:], ACTF.Sqrt, bias=epsT[:])
            nc.vector.reciprocal(var[:], var[:])
            nc.vector.tensor_tensor(alphaT, g_ap, var[:], ALU.mult)
            nc.vector.tensor_tensor(m2t[:], mean[:], alphaT, ALU.mult)
            nc.vector.tensor_sub(betaT, be_ap, m2t[:])

        # ====== phase B (stats) + classifier L1, interleaved by scheduler ===
        psB_s = top.enter_context(ExitStack())
        psB = psB_s.enter_context(tc.tile_pool(name="psB", bufs=1, space="PSUM"))
        sBx = top.enter_context(ExitStack())
        tmpP = sBx.enter_context(tc.tile_pool(name="tmpP", bufs=1))
        stP = sBx.enter_context(tc.tile_pool(name="stP", bufs=2))
        wkB = sBx.enter_context(tc.tile_pool(name="wkB", bufs=1))
        pcX = sBx.enter_context(tc.tile_pool(name="pcX", bufs=2))
        sC1 = top.enter_context(ExitStack())
        psC1 = sC1.enter_context(tc.tile_pool(name="psC1", bufs=4, space="PSUM"))

        # classifier L1 matmuls (independent of AR1) -> raw preacts into a1T
        for nb in range(4):
            xck = pcX.tile([128, 4, 512], F32, tag="xck")
            nc.sync.dma_start(xck[:], xt_d[:, :, nb * 512:(nb + 1) * 512])
            for jc in range(8):
                ps = psC1.tile([128, 512], F32, tag="c")
                for kt in range(4):
                    nc.tensor.matmul(ps[:], wc1_sb[:, kt, jc * 128:(jc + 1) * 128],
                                     xck[:, kt, :], start=(kt == 0), stop=(kt == 3))
                nc.vector.tensor_copy(a1T[:, jc, nb * 512:(nb + 1) * 512], ps[:])

        # classifier L1 stats (needs AR1), then bn+relu applied in place
        qs_c = stP.tile([128, 8, 2], F32, tag="qs")
        emit_l1_raw_stats(wc1_sb, None, 8, qs_c, False, (tmpP, psB))
        emit_alpha_beta(qs_c[:, :, 0], qs_c[:, :, 1],
                        vecl1_sb[:, 0:8, 0], vecl1_sb[:, 0:8, 1],
                        a1c[:], b1c[:], stP, (128, 8), "b")
        for jc in range(8):
            nc.scalar.activation(a1T[:, jc, :], a1T[:, jc, :], ACTF.Relu,
                                 scale=a1c[:, jc:jc + 1], bias=b1c[:, jc:jc + 1])

        # head L1 stats, sharded: this core computes only its 2 assigned
        # heads' raw (s, q); AllGather redistributes all 11.
        qs_m = stP.tile([128, 2, 8, 2], F32, tag="qsm")
        for slot in range(2):
            wkr = wkB.tile([128, 4, H1], F32R, tag="wkr")
            nc.sync.dma_start(wkr[:], rw1m_d[slot].bitcast(F32R))
            emit_l1_raw_stats(wkr[:].bitcast(F32), wkr, 8, qs_m[:, slot],
                              True, (tmpP, psB))
        nc.gpsimd.dma_start(qg_in[:], qs_m[:].rearrange("p s j t -> p (s j t)"))
        nc.gpsimd.collective_compute(
            "AllGather", ALU.bypass, replica_groups=RG,
            ins=[qg_in[:].opt()], outs=[qg_out[:].opt()])

        # gathered head L1 stats -> alpha/beta for all 11 heads (early, so
        # phase D's first evictions are never stats-blocked)
        qsH = stP.tile([128, KH, 8, 2], F32, tag="qsH")
        for k in range(KH):
            nc.sync.dma_start(
                qsH[:, k], qg_out[k % NCORES][:, (k // NCORES) * 16:
                                              (k // NCORES) * 16 + 16]
                .rearrange("p (j t) -> p j t", t=2))
        emit_alpha_beta(qsH[:, :, :, 0], qsH[:, :, :, 1],
                        vecl1_sb[:, 8:96, 0].rearrange("p (k j) -> p k j", k=KH),
                        vecl1_sb[:, 8:96, 1].rearrange("p (k j) -> p k j", k=KH),
                        A1H[:], B1H[:], stP, (128, KH, 8), "h")
        sC1.close()
        sBx.close()
        psB_s.close()
        sG.close()
        sW.close()

        # ================= phase C-L2: h2pre -> scratch + stats ============
        with ExitStack() as sC:
            psC = sC.enter_context(tc.tile_pool(name="psC", bufs=4, space="PSUM"))
            wstream = sC.enter_context(tc.tile_pool(name="wstream", bufs=3))
            h2pool = sC.enter_context(tc.tile_pool(name="h2pool", bufs=2))
            stC = sC.enter_context(tc.tile_pool(name="stC", bufs=4))
            for jc2 in range(16):
                wt2 = wstream.tile([128, 8, 128], F32, tag="wc2")
                nc.sync.dma_start(wt2[:], wc2_d[:, :, jc2 * 128:(jc2 + 1) * 128])
                h2row = h2pool.tile([128, 2048], F32, tag="h2row")
                mv6 = stC.tile([128, 4, 6], F32, tag="mv6")
                for nb in range(4):
                    ps = psC.tile([128, 512], F32, tag="c")
                    for kt in range(8):
                        nc.tensor.matmul(ps[:], wt2[:, kt, :],
                                         a1T[:, kt, nb * 512:(nb + 1) * 512],
                                         start=(kt == 0), stop=(kt == 7))
                    nc.vector.tensor_copy(h2row[:, nb * 512:(nb + 1) * 512], ps[:])
                    nc.vector.bn_stats(mv6[:, nb, :], h2row[:, nb * 512:(nb + 1) * 512])
                nc.sync.dma_start(h2s[:, jc2, :], h2row[:])
                mv = stC.tile([128, 2], F32, tag="mv")
                nc.vector.bn_aggr(mv[:], mv6[:])
                tq = stC.tile([128, 1], F32, tag="tq")
                nc.vector.tensor_tensor(tq[:], mv[:, 0:1], mv[:, 0:1], ALU.mult)
                nc.vector.tensor_add(tq[:], tq[:], mv[:, 1:2])
                nc.vector.tensor_scalar_mul(stats_cls[:, jc2, 1:2], tq[:], float(BC))
                nc.vector.tensor_scalar_mul(stats_cls[:, jc2, 0:1], mv[:, 0:1],
                                            float(BC))
        sA1.close()
        # classifier L2 stats AllReduce, overlapped with heads phase
        nc.gpsimd.dma_start(sc_in[:], stats_cls[:].rearrange("p c t -> p (c t)"))
        nc.gpsimd.collective_compute(
            "AllReduce", ALU.add, replica_groups=RG,
            ins=[sc_in[:].opt()], outs=[sc_out[:].opt()])
        # classifier alpha2/beta2 computed as soon as AR2a lands (scratch in
        # const pool so phase E's a2row work is never stats-blocked)
        statgc = const.tile([128, 16, 2], F32)
        nc.sync.dma_start(statgc[:].rearrange("p c t -> p (c t)"), sc_out[:])
        emit_alpha_beta(statgc[:, :, 0], statgc[:, :, 1],
                        vecl2_sb[:, 0:16, 0], vecl2_sb[:, 0:16, 1],
                        a2c[:], b2c[:], const, (128, 16), "e")

        # ================= phase D: 11 reg heads (f32r) =================
        with ExitStack() as sD:
            wk1 = sD.enter_context(tc.tile_pool(name="wk1", bufs=2))
            wk2 = sD.enter_context(tc.tile_pool(name="wk2", bufs=1))
            a1kP = sD.enter_context(tc.tile_pool(name="a1kP", bufs=1))
            psD = sD.enter_context(tc.tile_pool(name="psD", bufs=4, space="PSUM"))
            r2P = sD.enter_context(tc.tile_pool(name="r2P", bufs=2))
            stD = sD.enter_context(tc.tile_pool(name="stD", bufs=4))

            for k in range(KH):
                w1r = wk1.tile([128, 4, H1], F32R, tag="w1r")
                nc.sync.dma_start(w1r[:], rw1_d[k].bitcast(F32R))
                w2r = wk2.tile([128, 8, 512], F32R, tag="w2r")
                nc.sync.dma_start(w2r[:], rw2_d[k].bitcast(F32R))
                a1k = a1kP.tile([128, 8, 2048], F32R, tag="a1k")
                for jc in range(8):
                    for nb in range(4):
                        ps = psD.tile([128, 512], F32, tag="d")
                        for kt in range(4):
                            nc.tensor.matmul(ps[:], w1r[:, kt, jc * 128:(jc + 1) * 128],
                                             xtr_sb[:, kt, nb * 512:(nb + 1) * 512],
                                             start=(kt == 0), stop=(kt == 3))
                        nc.scalar.activation(a1k[:, jc, nb * 512:(nb + 1) * 512],
                                             ps[:], ACTF.Relu,
                                             scale=A1H[:, k, jc:jc + 1],
                                             bias=B1H[:, k, jc:jc + 1])
                for jc2 in range(4):
                    r2row = r2P.tile([128, 2048], mybir.dt.bfloat16, tag="r2row")
                    mv6 = stD.tile([128, 4, 6], F32, tag="mv6")
                    for nb in range(4):
                        ps = psD.tile([128, 512], F32, tag="d")
                        for kt in range(8):
                            nc.tensor.matmul(ps[:],
                                             w2r[:, kt, jc2 * 128:(jc2 + 1) * 128],
                                             a1k[:, kt, nb * 512:(nb + 1) * 512],
                                             start=(kt == 0), stop=(kt == 7))
                        nc.vector.tensor_copy(r2row[:, nb * 512:(nb + 1) * 512], ps[:])
                        nc.vector.bn_stats(mv6[:, nb, :],
                                           r2row[:, nb * 512:(nb + 1) * 512])
                    nc.sync.dma_start(r2s[k][:, jc2, :], r2row[:])
                    mv = stD.tile([128, 2], F32, tag="mv")
                    nc.vector.bn_aggr(mv[:], mv6[:])
                    tq = stD.tile([128, 1], F32, tag="tq")
                    nc.vector.tensor_tensor(tq[:], mv[:, 0:1], mv[:, 0:1], ALU.mult)
                    nc.vector.tensor_add(tq[:], tq[:], mv[:, 1:2])
                    col = 4 * k + jc2
                    nc.vector.tensor_scalar_mul(stats_hd[:, col, 1:2], tq[:],
                                                float(BC))
                    nc.vector.tensor_scalar_mul(stats_hd[:, col, 0:1], mv[:, 0:1],
                                                float(BC))
        nc.gpsimd.dma_start(sh_in[:], stats_hd[:].rearrange("p c t -> p (c t)"))
        nc.gpsimd.collective_compute(
            "AllReduce", ALU.add, replica_groups=RG,
            ins=[sh_in[:].opt()], outs=[sh_out[:].opt()])

        # ================= phase E: bn2 + logits/softmax + r3/sigmoid ======
        with ExitStack() as sE:
            stE = sE.enter_context(tc.tile_pool(name="stE", bufs=1))
            pE = sE.enter_context(tc.tile_pool(name="pE", bufs=3))
            psEF = sE.enter_context(tc.tile_pool(name="psEF", bufs=1, space="PSUM"))
            smE = sE.enter_context(tc.tile_pool(name="smE", bufs=2))

            # head alpha2/beta2 (AR2b)
            statgh = stE.tile([128, 44, 2], F32)
            nc.sync.dma_start(statgh[:].rearrange("p c t -> p (c t)"), sh_out[:])
            emit_alpha_beta(statgh[:, :, 0].rearrange("p (k j) -> p k j", k=KH),
                            statgh[:, :, 1].rearrange("p (k j) -> p k j", k=KH),
                            vecl2_sb[:, 16:60, 0].rearrange("p (k j) -> p k j", k=KH),
                            vecl2_sb[:, 16:60, 1].rearrange("p (k j) -> p k j", k=KH),
                            A2H[:], B2H[:], stE, (128, KH, 4), "f")

            rw3_sb = stE.tile([128, 4, KH], F32)
            nc.sync.dma_start(rw3_sb[:], rw3_d[:])
            wc3_sb = stE.tile([128, 16, KH], F32)
            nc.sync.dma_start(wc3_sb[:], wc3_d[:])

            # --- classifier first: it only waits on AR2a (already landed),
            # so it fills the AR2b latency and the head-DMA stalls ---
            lgT_ps = psEF.tile([11, 2048], F32, tag="lgT")
            for jc2 in range(16):
                a2row = pE.tile([128, 2048], F32, tag="a2row")
                nc.sync.dma_start(a2row[:], h2s[:, jc2, :])
                nc.scalar.activation(a2row[:], a2row[:], ACTF.Relu,
                                     scale=a2c[:, jc2:jc2 + 1],
                                     bias=b2c[:, jc2:jc2 + 1])
                for ch in range(4):
                    nc.tensor.matmul(lgT_ps[:, ch * 512:(ch + 1) * 512],
                                     wc3_sb[:, jc2, :],
                                     a2row[:, ch * 512:(ch + 1) * 512],
                                     start=(jc2 == 0), stop=(jc2 == 15))

            # --- heads: a2k = relu(bn(r2)), r3 = a2k . w3k, sigmoid ---
            for k in range(KH):
                a2kraw = pE.tile([128, 4, 2048], mybir.dt.bfloat16, tag="a2kraw")
                nc.sync.dma_start(a2kraw[:], r2s[k][:])
                a2k = pE.tile([128, 4, 2048], F32, tag="a2k", bufs=1)
                for jc2 in range(4):
                    nc.scalar.activation(a2k[:, jc2, :], a2kraw[:, jc2, :], ACTF.Relu,
                                         scale=A2H[:, k, jc2:jc2 + 1],
                                         bias=B2H[:, k, jc2:jc2 + 1])
                for bc in range(16):
                    ps = psEF.tile([128, 1], F32, tag=f"r3{bc % 2}")
                    for jc2 in range(4):
                        nc.tensor.matmul(ps[:],
                                         a2k[:, jc2, bc * 128:(bc + 1) * 128],
                                         rw3_sb[:, jc2, k:k + 1],
                                         start=(jc2 == 0), stop=(jc2 == 3))
                    nc.scalar.activation(pcts_sb[:, bc, k:k + 1], ps[:], ACTF.Sigmoid,
                                         bias=row11_sb[:, 1, k:k + 1])
            nc.sync.dma_start(op_d[:], pcts_sb[:])
            lgT_sb = stE.tile([11, 2048], F32)
            nc.vector.tensor_copy(lgT_sb[:], lgT_ps[:])
            for bc in range(16):
                psl = psEF.tile([128, KH], F32, tag=f"lg{bc % 2}")
                nc.tensor.matmul(psl[:], lgT_sb[:, bc * 128:(bc + 1) * 128],
                                 eye11_sb[:], start=True, stop=True)
                nc.vector.tensor_tensor(logits_sb[:, bc, :], psl[:],
                                        row11_sb[:, 0, :], ALU.add)
            nc.sync.dma_start(ol_d[:], logits_sb[:])

            # --- softmax over all 16x11 at once ---
            mx = smE.tile([128, 16, 1], F32, tag="mx")
            nc.vector.tensor_reduce(mx[:], logits_sb[:], AXX, ALU.max)
            et = smE.tile([128, 16, KH], F32, tag="et")
            nc.vector.tensor_tensor(et[:], logits_sb[:],
                                    mx[:].to_broadcast([128, 16, KH]), ALU.subtract)
            nc.scalar.activation(et[:], et[:], ACTF.Exp)
            sm = smE.tile([128, 16, 1], F32, tag="sm")
            nc.vector.tensor_reduce(sm[:], et[:], AXX, ALU.add)
            nc.vector.reciprocal(sm[:], sm[:])
            nc.vector.tensor_tensor(p_sb[:], et[:],
                                    sm[:].to_broadcast([128, 16, KH]), ALU.mult)

        # ================= phase F: combine (vectorized) =================
        with ExitStack() as sF:
            pG = sF.enter_context(tc.tile_pool(name="pG", bufs=1))
            S3 = [128, 16, KH]
            m8all = pG.tile([128, 16, 8], F32)
            for bc in range(16):
                nc.vector.max(m8all[:, bc, :], p_sb[:, bc, :])
            valid = pG.tile(S3, F32)
            nc.vector.tensor_scalar(valid[:], p_sb[:], THR, None, ALU.is_gt)
            hv = pG.tile([128, 16, 1], F32)
            nc.vector.tensor_reduce(hv[:], valid[:], AXX, ALU.max)
            top3 = pG.tile(S3, F32)
            nc.vector.tensor_tensor(top3[:], p_sb[:],
                                    m8all[:, :, 2:3].to_broadcast(S3), ALU.is_ge)
            eff = pG.tile(S3, F32)
            nc.vector.tensor_sub(eff[:], valid[:], top3[:])
            nc.vector.tensor_tensor(eff[:], eff[:], hv[:].to_broadcast(S3), ALU.mult)
            nc.vector.tensor_add(eff[:], eff[:], top3[:])
            w = pG.tile(S3, F32)
            nc.vector.tensor_tensor(w[:], p_sb[:], eff[:], ALU.mult)
            ws = pG.tile([128, 16, 1], F32)
            nc.vector.tensor_reduce(ws[:], w[:], AXX, ALU.add)
            nc.vector.reciprocal(ws[:], ws[:])
            atp = pG.tile(S3, F32)
            nc.vector.tensor_tensor(atp[:], pcts_sb[:],
                                    row11_sb[:, 2:3, :].to_broadcast(S3), ALU.add)
            nc.scalar.activation(atp[:], atp[:], ACTF.Exp, scale=LN10)
            nc.vector.tensor_tensor(w[:], w[:], atp[:], ALU.mult)
            num = pG.tile([128, 16, 1], F32)
            nc.vector.tensor_reduce(num[:], w[:], AXX, ALU.add)
            nc.vector.tensor_tensor(final_sb[:].rearrange("p (c o) -> p c o", o=1), num[:],
                                    ws[:], ALU.mult)
            sel = pG.tile(S3, F32)
            nc.vector.tensor_tensor(sel[:], p_sb[:],
                                    m8all[:, :, 0:1].to_broadcast(S3), ALU.is_ge)
            t2 = pG.tile(S3, F32)
            nc.vector.tensor_tensor(t2[:], pcts_sb[:], sel[:], ALU.mult)
            pm = pG.tile([128, 16, 1], F32)
            nc.vector.tensor_reduce(pm[:], t2[:], AXX, ALU.add)
            nc.vector.tensor_tensor(t2[:], row11_sb[:, 2:3, :].to_broadcast(S3),
                                    sel[:], ALU.mult)
            lm = pG.tile([128, 16, 1], F32)
            nc.vector.tensor_reduce(lm[:], t2[:], AXX, ALU.add)
            nc.vector.tensor_add(pm[:], pm[:], lm[:])
            nc.scalar.activation(atpm_sb[:].rearrange("p (c o) -> p c o", o=1), pm[:],
                                 ACTF.Exp, scale=LN10)
            nc.sync.dma_start(of_d[:], final_sb[:])
            nc.sync.dma_start(oa_d[:], atpm_sb[:])

    nc.compile()
    _BUILD_CACHE["nc"] = nc
    return nc


def _pack_kt(W, kt):
    # [kt*128, M] -> [128, kt, M]
    W = np.ascontiguousarray(np.asarray(W, np.float32))
    return np.ascontiguousarray(W.reshape(kt, 128, W.shape[1]).transpose(1, 0, 2))


def _pack_vec(v, t):
    # [t*128] -> [128, t]
    return np.ascontiguousarray(np.asarray(v, np.float32).reshape(t, 128).T)


def kernel(x, Wc1, bc1, gc1, bec1, Wc2, bc2, gc2, bec2, Wc3, bc3,
           Rw1, Rb1, Rg1, Rbe1, Rw2, Rb2, Rg2, Rbe2, Rw3, Rb3):
    nc = _build()
    f = np.float32
    x = np.asarray(x, f)

    wc1 = _pack_kt(Wc1, 4)
    wc2 = _pack_kt(Wc2, 8)
    wc3 = _pack_kt(Wc3, 16)
    rw1 = np.ascontiguousarray(np.stack([_pack_kt(np.asarray(Rw1)[k], 4)
                                         for k in range(KH)]))
    rw2 = np.ascontiguousarray(np.stack([_pack_kt(np.asarray(Rw2)[k], 8)
                                         for k in range(KH)]))
    rw3 = _pack_kt(np.asarray(Rw3, f)[:, :, 0].T, 4)     # [512, 11] -> [128,4,11]

    vecl1 = np.zeros((128, 96, 2), f)
    vecl1[:, 0:8, 0] = _pack_vec(gc1, 8)
    vecl1[:, 0:8, 1] = _pack_vec(bec1, 8)
    for k in range(KH):
        vecl1[:, 8 + 8 * k:16 + 8 * k, 0] = _pack_vec(np.asarray(Rg1)[k], 8)
        vecl1[:, 8 + 8 * k:16 + 8 * k, 1] = _pack_vec(np.asarray(Rbe1)[k], 8)
    vecl2 = np.zeros((128, 60, 2), f)
    vecl2[:, 0:16, 0] = _pack_vec(gc2, 16)
    vecl2[:, 0:16, 1] = _pack_vec(bec2, 16)
    for k in range(KH):
        vecl2[:, 16 + 4 * k:20 + 4 * k, 0] = _pack_vec(np.asarray(Rg2)[k], 4)
        vecl2[:, 16 + 4 * k:20 + 4 * k, 1] = _pack_vec(np.asarray(Rbe2)[k], 4)
    row11 = np.zeros((128, 4, KH), f)
    row11[:, 0, :] = np.asarray(bc3, f)
    row11[:, 1, :] = np.asarray(Rb3, f)[:, 0]
    row11[:, 2, :] = np.arange(KH, dtype=f) - 6.0
    eye11 = np.eye(KH, dtype=f)

    common = dict(wc1=wc1, wc2=wc2, wc3=wc3, rw1=rw1, rw2=rw2, rw3=rw3,
                  vecl1=vecl1, vecl2=vecl2, row11=row11, eye11=eye11)
    in_maps = []
    for c in range(NCORES):
        xc = x[c * BC:(c + 1) * BC]                      # [2048, 512]
        xn = np.ascontiguousarray(xc.reshape(16, 128, 512).transpose(1, 0, 2))
        xt = np.ascontiguousarray(xc.T.reshape(4, 128, 2048).transpose(1, 0, 2))
        k0, k1 = c, c + NCORES if c + NCORES < KH else c
        rw1m = np.ascontiguousarray(np.stack([rw1[k0], rw1[k1]]))
        in_maps.append(dict(common, xn=xn, xt=xt, rw1m=rw1m))

    res = run_bass_kernel_spmd(nc, in_maps, core_ids=list(range(NCORES)),
                               trace=TRACE)
    global LAST_RESULT
    LAST_RESULT = res

    finals, logits, pcts, atpms = [], [], [], []
    for c in range(NCORES):
        r = res.results[c]
        finals.append(r["of"].T.reshape(BC, 1))
        logits.append(r["ol"].transpose(1, 0, 2).reshape(BC, KH))
        pcts.append(r["op"].transpose(1, 0, 2).reshape(BC, KH))
        atpms.append(r["oa"].T.reshape(BC, 1))
    return (np.ascontiguousarray(np.concatenate(finals)),
            np.ascontiguousarray(np.concatenate(logits)),
            np.ascontiguousarray(np.concatenate(pcts)),
            np.ascontiguousarray(np.concatenate(atpms)))


# revision 31
# speedup vs baseline: 10045.2310x; 10045.2310x over previous
import sys
import numpy as np

sys.path.insert(0, "/opt/trn_rl_repo")

import concourse.bass as bass  # noqa: E402
import concourse.mybir as mybir  # noqa: E402
import concourse.tile as tile  # noqa: E402
from concourse import bacc  # noqa: E402
from concourse.bass_utils import run_bass_kernel_spmd  # noqa: E402
from contextlib import ExitStack  # noqa: E402

F32 = mybir.dt.float32
F32R = mybir.dt.float32r
ACTF = mybir.ActivationFunctionType
ALU = mybir.AluOpType
AXX = mybir.AxisListType.X

NCORES = 8
B, D, H1, H2, KH = 16384, 512, 1024, 2048, 11
BC = B // NCORES            # 2048 rows per core
EPS = 1e-5
THR = 0.25
LN10 = float(np.log(10.0))
N_INV = 1.0 / float(B)

_BUILD_CACHE = {}
TRACE = False          # test harness hook: set True to capture a profile
LAST_RESULT = None     # test harness hook: BassKernelResults of last run


def _build():
    if "nc" in _BUILD_CACHE:
        return _BUILD_CACHE["nc"]
    nc = bacc.Bacc(None, target_bir_lowering=False, debug=False)

    # ------- external I/O (per core) -------
    xn_d = nc.dram_tensor("xn", [128, 16, 512], F32, kind="ExternalInput")
    xt_d = nc.dram_tensor("xt", [128, 4, 2048], F32, kind="ExternalInput")
    wc1_d = nc.dram_tensor("wc1", [128, 4, H1], F32, kind="ExternalInput")
    wc2_d = nc.dram_tensor("wc2", [128, 8, H2], F32, kind="ExternalInput")
    wc3_d = nc.dram_tensor("wc3", [128, 16, KH], F32, kind="ExternalInput")
    rw1_d = nc.dram_tensor("rw1", [KH, 128, 4, H1], F32, kind="ExternalInput")
    rw1m_d = nc.dram_tensor("rw1m", [2, 128, 4, H1], F32, kind="ExternalInput")
    rw2_d = nc.dram_tensor("rw2", [KH, 128, 8, 512], F32, kind="ExternalInput")
    rw3_d = nc.dram_tensor("rw3", [128, 4, KH], F32, kind="ExternalInput")
    vecl1_d = nc.dram_tensor("vecl1", [128, 96, 2], F32, kind="ExternalInput")
    vecl2_d = nc.dram_tensor("vecl2", [128, 60, 2], F32, kind="ExternalInput")
    row11_d = nc.dram_tensor("row11", [128, 4, KH], F32, kind="ExternalInput")
    eye11_d = nc.dram_tensor("eye11", [11, 11], F32, kind="ExternalInput")

    of_d = nc.dram_tensor("of", [128, 16], F32, kind="ExternalOutput")
    ol_d = nc.dram_tensor("ol", [128, 16, KH], F32, kind="ExternalOutput")
    op_d = nc.dram_tensor("op", [128, 16, KH], F32, kind="ExternalOutput")
    oa_d = nc.dram_tensor("oa", [128, 16], F32, kind="ExternalOutput")

    # ------- internal DRAM -------
    gin = nc.dram_tensor("gin", [128, 2052], F32, kind="Internal")
    gout = nc.dram_tensor("gout", [128, 2052], F32, kind="Internal",
                          addr_space="Shared")
    qg_in = nc.dram_tensor("qg_in", [128, 32], F32, kind="Internal")
    qg_out = nc.dram_tensor("qg_out", [8, 128, 32], F32, kind="Internal",
                            addr_space="Shared")
    sc_in = nc.dram_tensor("sc_in", [128, 32], F32, kind="Internal")
    sc_out = nc.dram_tensor("sc_out", [128, 32], F32, kind="Internal",
                            addr_space="Shared")
    sh_in = nc.dram_tensor("sh_in", [128, 88], F32, kind="Internal")
    sh_out = nc.dram_tensor("sh_out", [128, 88], F32, kind="Internal",
                            addr_space="Shared")
    h2s = nc.dram_tensor("h2s", [128, 16, 2048], F32, kind="Internal")
    # r2 preacts round-trip in fp16 (range is tiny, |r2| < ~10): only the
    # smooth pcts path flows through them, and halving this 92MB of traffic
    # matters more than the ~2^-11 rounding it adds there.
    r2s = nc.dram_tensor("r2s", [KH, 128, 4, 2048], mybir.dt.float16,
                         kind="Internal")

    RG = [list(range(NCORES))]

    with tile.TileContext(nc) as tc, ExitStack() as top:
        const = top.enter_context(tc.tile_pool(name="const", bufs=1))
        # long-lived big tiles; strict LIFO pool discipline, so the pools
        # that close earliest open last.
        pxr = top.enter_context(tc.tile_pool(name="pxr", bufs=1))
        sA1 = top.enter_context(ExitStack())      # a1T: closed after C-L2
        a1pool = sA1.enter_context(tc.tile_pool(name="a1pool", bufs=1))
        sW = top.enter_context(ExitStack())       # wc1: closed after C-L1
        pcW = sW.enter_context(tc.tile_pool(name="pcW", bufs=1))
        sG = top.enter_context(ExitStack())       # G: closed after B
        gpool = sG.enter_context(tc.tile_pool(name="gpool", bufs=1))
        pa_top = top.enter_context(ExitStack())   # xn: closed after A
        pa = pa_top.enter_context(tc.tile_pool(name="pa", bufs=1))

        # x natural-layout shard loads first (per-tile chunks so the gram
        # matmuls pipeline with the loads)
        xn_sb = pa.tile([128, 16, 512], F32)
        for t in range(16):
            nc.sync.dma_start(xn_sb[:, t, :], xn_d[:, t, :])

        ones = const.tile([128, 1], F32)
        nc.vector.memset(ones, 1.0)
        epsT = const.tile([128, 1], F32)
        nc.vector.memset(epsT, EPS)
        row11_sb = const.tile([128, 4, KH], F32)
        nc.sync.dma_start(row11_sb[:], row11_d[:])
        vecl1_sb = const.tile([128, 96, 2], F32)
        nc.sync.dma_start(vecl1_sb[:], vecl1_d[:])
        vecl2_sb = const.tile([128, 60, 2], F32)
        nc.sync.dma_start(vecl2_sb[:], vecl2_d[:])
        eye11_sb = const.tile([11, 11], F32)
        nc.sync.dma_start(eye11_sb[:], eye11_d[:])
        stats_cls = const.tile([128, 16, 2], F32)
        stats_hd = const.tile([128, 44, 2], F32)
        a1c = const.tile([128, 8], F32)
        b1c = const.tile([128, 8], F32)
        A1H = const.tile([128, KH, 8], F32)
        B1H = const.tile([128, KH, 8], F32)
        a2c = const.tile([128, 16], F32)
        b2c = const.tile([128, 16], F32)
        A2H = const.tile([128, KH, 4], F32)
        B2H = const.tile([128, KH, 4], F32)
        p_sb = const.tile([128, 16, KH], F32)
        pcts_sb = const.tile([128, 16, KH], F32)
        logits_sb = const.tile([128, 16, KH], F32)
        final_sb = const.tile([128, 16], F32)
        atpm_sb = const.tile([128, 16], F32)

        xtr_sb = pxr.tile([128, 4, 2048], F32R)
        nc.sync.dma_start(xtr_sb[:], xt_d[:].bitcast(F32R))
        a1T = a1pool.tile([128, 8, 2048], F32)
        wc1_sb = pcW.tile([128, 4, H1], F32)
        nc.sync.dma_start(wc1_sb[:], wc1_d[:])
        G32g = gpool.tile([128, 4, 512], F32)
        Gr = gpool.tile([128, 4, 512], F32R)
        csg = gpool.tile([128, 4], F32)

        # ================= phase A: Gram + colsum =================
        with ExitStack() as sA:
            psA = sA.enter_context(tc.tile_pool(name="psA", bufs=1, space="PSUM"))
            G32 = pa.tile([128, 4, 512], F32)
            cs0 = pa.tile([128, 4], F32)
            psgs = [psA.tile([128, 512], F32, tag=f"g{m}", name=f"psg{m}")
                    for m in range(4)]
            pscs = [psA.tile([128, 1], F32, tag=f"c{m}", name=f"psc{m}")
                    for m in range(4)]
            for t in range(16):
                for m in range(4):
                    nc.tensor.matmul(psgs[m][:], xn_sb[:, t, m * 128:(m + 1) * 128],
                                     xn_sb[:, t, :], start=(t == 0), stop=(t == 15))
                    nc.tensor.matmul(pscs[m][:], xn_sb[:, t, m * 128:(m + 1) * 128],
                                     ones[:], start=(t == 0), stop=(t == 15))
            for m in range(4):
                nc.vector.tensor_copy(G32[:, m, :], psgs[m][:])
                nc.vector.tensor_copy(cs0[:, m:m + 1], pscs[m][:])
            nc.gpsimd.dma_start(gin[:, 0:2048], G32[:].rearrange("p m d -> p (m d)"))
            nc.gpsimd.dma_start(gin[:, 2048:2052], cs0[:])
        pa_top.close()

        nc.gpsimd.collective_compute(
            "AllReduce", ALU.add, replica_groups=RG,
            ins=[gin[:].opt()], outs=[gout[:].opt()])
        nc.sync.dma_start(G32g[:].rearrange("p m d -> p (m d)"), gout[:, 0:2048])
        nc.sync.dma_start(Gr[:].rearrange("p m d -> p (m d)"),
                          gout[:, 0:2048].bitcast(F32R))
        nc.sync.dma_start(csg[:], gout[:, 2048:2052])

        # stats helper: for a first-layer weight W [512, H] compute raw
        #   s_j = (W^T cs)_j            (sum of h over full batch)
        #   q_j = sum_d1 W.(G@W) [.,j]  (sum of h^2 over full batch)
        def emit_l1_raw_stats(W32, Wr, n_jt, qs_out, use_f32r, pools):
            tmpP, psP = pools
            Hn = n_jt * 128
            tmp = tmpP.tile([128, 4, Hn], F32, tag="tmp")
            for m4 in range(4):
                for nch in range(Hn // 512):
                    psgw = psP.tile([128, 512], F32, tag="gw")
                    for kt in range(4):
                        nc.tensor.matmul(
                            psgw[:],
                            (Gr if use_f32r else G32g)[:, kt, m4 * 128:(m4 + 1) * 128],
                            (Wr if use_f32r else W32)[:, kt, nch * 512:(nch + 1) * 512],
                            start=(kt == 0), stop=(kt == 3))
                    nc.vector.tensor_tensor(
                        tmp[:, m4, nch * 512:(nch + 1) * 512], psgw[:],
                        W32[:, m4, nch * 512:(nch + 1) * 512], ALU.mult)
            for jc in range(n_jt):
                psq = psP.tile([128, 1], F32, tag="q")
                for m4 in range(4):
                    nc.tensor.matmul(psq[:], tmp[:, m4, jc * 128:(jc + 1) * 128],
                                     ones[:], start=(m4 == 0), stop=(m4 == 3))
                pss = psP.tile([128, 1], F32, tag="s")
                for kt in range(4):
                    nc.tensor.matmul(pss[:], W32[:, kt, jc * 128:(jc + 1) * 128],
                                     csg[:, kt:kt + 1], start=(kt == 0), stop=(kt == 3))
                nc.vector.tensor_copy(qs_out[:, jc, 0:1], pss[:])
                nc.vector.tensor_copy(qs_out[:, jc, 1:2], psq[:])

        # raw (s, q) -> bn scale/shift: alpha = g*rsqrt(var+eps),
        # beta = be - mean*alpha.  All [128, ...] elementwise; sq[..., 0]=s,
        # sq[..., 1]=q; g_be[..., 0]=gamma, [..., 1]=beta_in.
        def emit_alpha_beta(s_ap, q_ap, g_ap, be_ap, alphaT, betaT, scr, shp, tg):
            mean = scr.tile(list(shp), F32, tag=f"mean{tg}")
            var = scr.tile(list(shp), F32, tag=f"var{tg}")
            m2t = scr.tile(list(shp), F32, tag=f"m2t{tg}")
            nc.vector.tensor_scalar_mul(mean[:], s_ap, N_INV)
            nc.vector.tensor_scalar_mul(var[:], q_ap, N_INV)
            nc.vector.tensor_tensor(m2t[:], mean[:], mean[:], ALU.mult)
            nc.vector.tensor_sub(var[:], var[:], m2t[:])
            nc.scalar.activation(var[:], var[:], ACTF.Sqrt, bias=epsT[:])
            nc.vector.reciprocal(var[:], var[:])
            nc.vector.tensor_tensor(alphaT, g_ap, var[:], ALU.mult)
            nc.vector.tensor_tensor(m2t[:], mean[:], alphaT, ALU.mult)
            nc.vector.tensor_sub(betaT, be_ap, m2t[:])

        # ====== phase B (stats) + classifier L1, interleaved by scheduler ===
        psB_s = top.enter_context(ExitStack())
        psB = psB_s.enter_context(tc.tile_pool(name="psB", bufs=1, space="PSUM"))
        sBx = top.enter_context(ExitStack())
        tmpP = sBx.enter_context(tc.tile_pool(name="tmpP", bufs=1))
        stP = sBx.enter_context(tc.tile_pool(name="stP", bufs=2))
        wkB = sBx.enter_context(tc.tile_pool(name="wkB", bufs=1))
        pcX = sBx.enter_context(tc.tile_pool(name="pcX", bufs=2))
        sC1 = top.enter_context(ExitStack())
        psC1 = sC1.enter_context(tc.tile_pool(name="psC1", bufs=4, space="PSUM"))

        # classifier L1 matmuls (independent of AR1) -> raw preacts into a1T
        for nb in range(4):
            xck = pcX.tile([128, 4, 512], F32, tag="xck")
            nc.sync.dma_start(xck[:], xt_d[:, :, nb * 512:(nb + 1) * 512])
            for jc in range(8):
                ps = psC1.tile([128, 512], F32, tag="c")
                for kt in range(4):
                    nc.tensor.matmul(ps[:], wc1_sb[:, kt, jc * 128:(jc + 1) * 128],
                                     xck[:, kt, :], start=(kt == 0), stop=(kt == 3))
                nc.vector.tensor_copy(a1T[:, jc, nb * 512:(nb + 1) * 512], ps[:])

        # classifier L1 stats (needs AR1), then bn+relu applied in place
        qs_c = stP.tile([128, 8, 2], F32, tag="qs")
        emit_l1_raw_stats(wc1_sb, None, 8, qs_c, False, (tmpP, psB))
        emit_alpha_beta(qs_c[:, :, 0], qs_c[:, :, 1],
                        vecl1_sb[:, 0:8, 0], vecl1_sb[:, 0:8, 1],
                        a1c[:], b1c[:], stP, (128, 8), "b")
        for jc in range(8):
            nc.scalar.activation(a1T[:, jc, :], a1T[:, jc, :], ACTF.Relu,
                                 scale=a1c[:, jc:jc + 1], bias=b1c[:, jc:jc + 1])

        # head L1 stats, sharded: this core computes only its 2 assigned
        # heads' raw (s, q); AllGather redistributes all 11.
        qs_m = stP.tile([128, 2, 8, 2], F32, tag="qsm")
        for slot in range(2):
            wkr = wkB.tile([128, 4, H1], F32R, tag="wkr")
            nc.sync.dma_start(wkr[:], rw1m_d[slot].bitcast(F32R))
            emit_l1_raw_stats(wkr[:].bitcast(F32), wkr, 8, qs_m[:, slot],
                              True, (tmpP, psB))
        nc.gpsimd.dma_start(qg_in[:], qs_m[:].rearrange("p s j t -> p (s j t)"))
        nc.gpsimd.collective_compute(
            "AllGather", ALU.bypass, replica_groups=RG,
            ins=[qg_in[:].opt()], outs=[qg_out[:].opt()])

        # gathered head L1 stats -> alpha/beta for all 11 heads (early, so
        # phase D's first evictions are never stats-blocked)
        qsH = stP.tile([128, KH, 8, 2], F32, tag="qsH")
        for k in range(KH):
            nc.sync.dma_start(
                qsH[:, k], qg_out[k % NCORES][:, (k // NCORES) * 16:
                                              (k // NCORES) * 16 + 16]
                .rearrange("p (j t) -> p j t", t=2))
        emit_alpha_beta(qsH[:, :, :, 0], qsH[:, :, :, 1],
                        vecl1_sb[:, 8:96, 0].rearrange("p (k j) -> p k j", k=KH),
                        vecl1_sb[:, 8:96, 1].rearrange("p (k j) -> p k j", k=KH),
                        A1H[:], B1H[:], stP, (128, KH, 8), "h")
        sC1.close()
        sBx.close()
        psB_s.close()
        sG.close()
        sW.close()

        # ================= phase C-L2: h2pre -> scratch + stats ============
        with ExitStack() as sC:
            psC = sC.enter_context(tc.tile_pool(name="psC", bufs=4, space="PSUM"))
            wstream = sC.enter_context(tc.tile_pool(name="wstream", bufs=3))
            h2pool = sC.enter_context(tc.tile_pool(name="h2pool", bufs=2))
            stC = sC.enter_context(tc.tile_pool(name="stC", bufs=4))
            for jc2 in range(16):
                wt2 = wstream.tile([128, 8, 128], F32, tag="wc2")
                nc.sync.dma_start(wt2[:], wc2_d[:, :, jc2 * 128:(jc2 + 1) * 128])
                h2row = h2pool.tile([128, 2048], F32, tag="h2row")
                mv6 = stC.tile([128, 4, 6], F32, tag="mv6")
                for nb in range(4):
                    ps = psC.tile([128, 512], F32, tag="c")
                    for kt in range(8):
                        nc.tensor.matmul(ps[:], wt2[:, kt, :],
                                         a1T[:, kt, nb * 512:(nb + 1) * 512],
                                         start=(kt == 0), stop=(kt == 7))
                    nc.vector.tensor_copy(h2row[:, nb * 512:(nb + 1) * 512], ps[:])
                    nc.vector.bn_stats(mv6[:, nb, :], h2row[:, nb * 512:(nb + 1) * 512])
                nc.sync.dma_start(h2s[:, jc2, :], h2row[:])
                mv = stC.tile([128, 2], F32, tag="mv")
                nc.vector.bn_aggr(mv[:], mv6[:])
                tq = stC.tile([128, 1], F32, tag="tq")
                nc.vector.tensor_tensor(tq[:], mv[:, 0:1], mv[:, 0:1], ALU.mult)
                nc.vector.tensor_add(tq[:], tq[:], mv[:, 1:2])
                nc.vector.tensor_scalar_mul(stats_cls[:, jc2, 1:2], tq[:], float(BC))
                nc.vector.tensor_scalar_mul(stats_cls[:, jc2, 0:1], mv[:, 0:1],
                                            float(BC))
        sA1.close()
        # classifier L2 stats AllReduce, overlapped with heads phase
        nc.gpsimd.dma_start(sc_in[:], stats_cls[:].rearrange("p c t -> p (c t)"))
        nc.gpsimd.collective_compute(
            "AllReduce", ALU.add, replica_groups=RG,
            ins=[sc_in[:].opt()], outs=[sc_out[:].opt()])
        # classifier alpha2/beta2 computed as soon as AR2a lands (scratch in
        # const pool so phase E's a2row work is never stats-blocked)
        statgc = const.tile([128, 16, 2], F32)
        nc.sync.dma_start(statgc[:].rearrange("p c t -> p (c t)"), sc_out[:])
        emit_alpha_beta(statgc[:, :, 0], statgc[:, :, 1],
                        vecl2_sb[:, 0:16, 0], vecl2_sb[:, 0:16, 1],
                        a2c[:], b2c[:], const, (128, 16), "e")

        # ================= phase D: 11 reg heads (f32r) =================
        with ExitStack() as sD:
            wk1 = sD.enter_context(tc.tile_pool(name="wk1", bufs=2))
            wk2 = sD.enter_context(tc.tile_pool(name="wk2", bufs=1))
            a1kP = sD.enter_context(tc.tile_pool(name="a1kP", bufs=1))
            psD = sD.enter_context(tc.tile_pool(name="psD", bufs=4, space="PSUM"))
            r2P = sD.enter_context(tc.tile_pool(name="r2P", bufs=2))
            stD = sD.enter_context(tc.tile_pool(name="stD", bufs=4))

            for k in range(KH):
                w1r = wk1.tile([128, 4, H1], F32R, tag="w1r")
                nc.sync.dma_start(w1r[:], rw1_d[k].bitcast(F32R))
                w2r = wk2.tile([128, 8, 512], F32R, tag="w2r")
                nc.sync.dma_start(w2r[:], rw2_d[k].bitcast(F32R))
                a1k = a1kP.tile([128, 8, 2048], F32R, tag="a1k")
                for jc in range(8):
                    for nb in range(4):
                        ps = psD.tile([128, 512], F32, tag="d")
                        for kt in range(4):
                            nc.tensor.matmul(ps[:], w1r[:, kt, jc * 128:(jc + 1) * 128],
                                             xtr_sb[:, kt, nb * 512:(nb + 1) * 512],
                                             start=(kt == 0), stop=(kt == 3))
                        nc.scalar.activation(a1k[:, jc, nb * 512:(nb + 1) * 512],
                                             ps[:], ACTF.Relu,
                                             scale=A1H[:, k, jc:jc + 1],
                                             bias=B1H[:, k, jc:jc + 1])
                for jc2 in range(4):
                    r2row = r2P.tile([128, 2048], mybir.dt.float16, tag="r2row")
                    mv6 = stD.tile([128, 4, 6], F32, tag="mv6")
                    for nb in range(4):
                        ps = psD.tile([128, 512], F32, tag="d")
                        for kt in range(8):
                            nc.tensor.matmul(ps[:],
                                             w2r[:, kt, jc2 * 128:(jc2 + 1) * 128],
                                             a1k[:, kt, nb * 512:(nb + 1) * 512],
                                             start=(kt == 0), stop=(kt == 7))
                        nc.vector.tensor_copy(r2row[:, nb * 512:(nb + 1) * 512], ps[:])
                        nc.vector.bn_stats(mv6[:, nb, :],
                                           r2row[:, nb * 512:(nb + 1) * 512])
                    nc.sync.dma_start(r2s[k][:, jc2, :], r2row[:])
                    mv = stD.tile([128, 2], F32, tag="mv")
                    nc.vector.bn_aggr(mv[:], mv6[:])
                    tq = stD.tile([128, 1], F32, tag="tq")
                    nc.vector.tensor_tensor(tq[:], mv[:, 0:1], mv[:, 0:1], ALU.mult)
                    nc.vector.tensor_add(tq[:], tq[:], mv[:, 1:2])
                    col = 4 * k + jc2
                    nc.vector.tensor_scalar_mul(stats_hd[:, col, 1:2], tq[:],
                                                float(BC))
                    nc.vector.tensor_scalar_mul(stats_hd[:, col, 0:1], mv[:, 0:1],
                                                float(BC))
        nc.gpsimd.dma_start(sh_in[:], stats_hd[:].rearrange("p c t -> p (c t)"))
        nc.gpsimd.collective_compute(
            "AllReduce", ALU.add, replica_groups=RG,
            ins=[sh_in[:].opt()], outs=[sh_out[:].opt()])

        # ================= phase E: bn2 + logits/softmax + r3/sigmoid ======
        with ExitStack() as sE:
            stE = sE.enter_context(tc.tile_pool(name="stE", bufs=1))
            pE = sE.enter_context(tc.tile_pool(name="pE", bufs=3))
            psEF = sE.enter_context(tc.tile_pool(name="psEF", bufs=1, space="PSUM"))
            smE = sE.enter_context(tc.tile_pool(name="smE", bufs=2))

            # head alpha2/beta2 (AR2b)
            statgh = stE.tile([128, 44, 2], F32)
            nc.sync.dma_start(statgh[:].rearrange("p c t -> p (c t)"), sh_out[:])
            emit_alpha_beta(statgh[:, :, 0].rearrange("p (k j) -> p k j", k=KH),
                            statgh[:, :, 1].rearrange("p (k j) -> p k j", k=KH),
                            vecl2_sb[:, 16:60, 0].rearrange("p (k j) -> p k j", k=KH),
                            vecl2_sb[:, 16:60, 1].rearrange("p (k j) -> p k j", k=KH),
                            A2H[:], B2H[:], stE, (128, KH, 4), "f")

            rw3_sb = stE.tile([128, 4, KH], F32)
            nc.sync.dma_start(rw3_sb[:], rw3_d[:])
            wc3_sb = stE.tile([128, 16, KH], F32)
            nc.sync.dma_start(wc3_sb[:], wc3_d[:])

            # --- classifier first: it only waits on AR2a (already landed),
            # so it fills the AR2b latency and the head-DMA stalls ---
            lgT_ps = psEF.tile([11, 2048], F32, tag="lgT")
            for jc2 in range(16):
                a2row = pE.tile([128, 2048], F32, tag="a2row")
                nc.sync.dma_start(a2row[:], h2s[:, jc2, :])
                nc.scalar.activation(a2row[:], a2row[:], ACTF.Relu,
                                     scale=a2c[:, jc2:jc2 + 1],
                                     bias=b2c[:, jc2:jc2 + 1])
                for ch in range(4):
                    nc.tensor.matmul(lgT_ps[:, ch * 512:(ch + 1) * 512],
                                     wc3_sb[:, jc2, :],
                                     a2row[:, ch * 512:(ch + 1) * 512],
                                     start=(jc2 == 0), stop=(jc2 == 15))

            # --- heads: a2k = relu(bn(r2)), r3 = a2k . w3k, sigmoid ---
            for k in range(KH):
                a2kraw = pE.tile([128, 4, 2048], mybir.dt.float16, tag="a2kraw")
                nc.sync.dma_start(a2kraw[:], r2s[k][:])
                a2k = pE.tile([128, 4, 2048], F32, tag="a2k", bufs=1)
                for jc2 in range(4):
                    nc.scalar.activation(a2k[:, jc2, :], a2kraw[:, jc2, :], ACTF.Relu,
                                         scale=A2H[:, k, jc2:jc2 + 1],
                                         bias=B2H[:, k, jc2:jc2 + 1])
                for bc in range(16):
                    ps = psEF.tile([128, 1], F32, tag=f"r3{bc % 2}")
                    for jc2 in range(4):
                        nc.tensor.matmul(ps[:],
                                         a2k[:, jc2, bc * 128:(bc + 1) * 128],
                                         rw3_sb[:, jc2, k:k + 1],
                                         start=(jc2 == 0), stop=(jc2 == 3))
                    nc.scalar.activation(pcts_sb[:, bc, k:k + 1], ps[:], ACTF.Sigmoid,
                                         bias=row11_sb[:, 1, k:k + 1])
            nc.sync.dma_start(op_d[:], pcts_sb[:])
            lgT_sb = stE.tile([11, 2048], F32)
            nc.vector.tensor_copy(lgT_sb[:], lgT_ps[:])
            for bc in range(16):
                psl = psEF.tile([128, KH], F32, tag=f"lg{bc % 2}")
                nc.tensor.matmul(psl[:], lgT_sb[:, bc * 128:(bc + 1) * 128],
                                 eye11_sb[:], start=True, stop=True)
                nc.vector.tensor_tensor(logits_sb[:, bc, :], psl[:],
                                        row11_sb[:, 0, :], ALU.add)
            nc.sync.dma_start(ol_d[:], logits_sb[:])

            # --- softmax over all 16x11 at once ---
            mx = smE.tile([128, 16, 1], F32, tag="mx")
            nc.vector.tensor_reduce(mx[:], logits_sb[:], AXX, ALU.max)
            et = smE.tile([128, 16, KH], F32, tag="et")
            nc.vector.tensor_tensor(et[:], logits_sb[:],
                                    mx[:].to_broadcast([128, 16, KH]), ALU.subtract)
            nc.scalar.activation(et[:], et[:], ACTF.Exp)
            sm = smE.tile([128, 16, 1], F32, tag="sm")
            nc.vector.tensor_reduce(sm[:], et[:], AXX, ALU.add)
            nc.vector.reciprocal(sm[:], sm[:])
            nc.vector.tensor_tensor(p_sb[:], et[:],
                                    sm[:].to_broadcast([128, 16, KH]), ALU.mult)

        # ================= phase F: combine (vectorized) =================
        with ExitStack() as sF:
            pG = sF.enter_context(tc.tile_pool(name="pG", bufs=1))
            S3 = [128, 16, KH]
            m8all = pG.tile([128, 16, 8], F32)
            for bc in range(16):
                nc.vector.max(m8all[:, bc, :], p_sb[:, bc, :])
            valid = pG.tile(S3, F32)
            nc.vector.tensor_scalar(valid[:], p_sb[:], THR, None, ALU.is_gt)
            hv = pG.tile([128, 16, 1], F32)
            nc.vector.tensor_reduce(hv[:], valid[:], AXX, ALU.max)
            top3 = pG.tile(S3, F32)
            nc.vector.tensor_tensor(top3[:], p_sb[:],
                                    m8all[:, :, 2:3].to_broadcast(S3), ALU.is_ge)
            eff = pG.tile(S3, F32)
            nc.vector.tensor_sub(eff[:], valid[:], top3[:])
            nc.vector.tensor_tensor(eff[:], eff[:], hv[:].to_broadcast(S3), ALU.mult)
            nc.vector.tensor_add(eff[:], eff[:], top3[:])
            w = pG.tile(S3, F32)
            nc.vector.tensor_tensor(w[:], p_sb[:], eff[:], ALU.mult)
            ws = pG.tile([128, 16, 1], F32)
            nc.vector.tensor_reduce(ws[:], w[:], AXX, ALU.add)
            nc.vector.reciprocal(ws[:], ws[:])
            atp = pG.tile(S3, F32)
            nc.vector.tensor_tensor(atp[:], pcts_sb[:],
                                    row11_sb[:, 2:3, :].to_broadcast(S3), ALU.add)
            nc.scalar.activation(atp[:], atp[:], ACTF.Exp, scale=LN10)
            nc.vector.tensor_tensor(w[:], w[:], atp[:], ALU.mult)
            num = pG.tile([128, 16, 1], F32)
            nc.vector.tensor_reduce(num[:], w[:], AXX, ALU.add)
            nc.vector.tensor_tensor(final_sb[:].rearrange("p (c o) -> p c o", o=1), num[:],
                                    ws[:], ALU.mult)
            sel = pG.tile(S3, F32)
            nc.vector.tensor_tensor(sel[:], p_sb[:],
                                    m8all[:, :, 0:1].to_broadcast(S3), ALU.is_ge)
            t2 = pG.tile(S3, F32)
            nc.vector.tensor_tensor(t2[:], pcts_sb[:], sel[:], ALU.mult)
            pm = pG.tile([128, 16, 1], F32)
            nc.vector.tensor_reduce(pm[:], t2[:], AXX, ALU.add)
            nc.vector.tensor_tensor(t2[:], row11_sb[:, 2:3, :].to_broadcast(S3),
                                    sel[:], ALU.mult)
            lm = pG.tile([128, 16, 1], F32)
            nc.vector.tensor_reduce(lm[:], t2[:], AXX, ALU.add)
            nc.vector.tensor_add(pm[:], pm[:], lm[:])
            nc.scalar.activation(atpm_sb[:].rearrange("p (c o) -> p c o", o=1), pm[:],
                                 ACTF.Exp, scale=LN10)
            nc.sync.dma_start(of_d[:], final_sb[:])
            nc.sync.dma_start(oa_d[:], atpm_sb[:])

    nc.compile()
    _BUILD_CACHE["nc"] = nc
    return nc


def _pack_kt(W, kt):
    # [kt*128, M] -> [128, kt, M]
    W = np.ascontiguousarray(np.asarray(W, np.float32))
    return np.ascontiguousarray(W.reshape(kt, 128, W.shape[1]).transpose(1, 0, 2))


def _pack_vec(v, t):
    # [t*128] -> [128, t]
    return np.ascontiguousarray(np.asarray(v, np.float32).reshape(t, 128).T)


def kernel(x, Wc1, bc1, gc1, bec1, Wc2, bc2, gc2, bec2, Wc3, bc3,
           Rw1, Rb1, Rg1, Rbe1, Rw2, Rb2, Rg2, Rbe2, Rw3, Rb3):
    nc = _build()
    f = np.float32
    x = np.asarray(x, f)

    wc1 = _pack_kt(Wc1, 4)
    wc2 = _pack_kt(Wc2, 8)
    wc3 = _pack_kt(Wc3, 16)
    rw1 = np.ascontiguousarray(np.stack([_pack_kt(np.asarray(Rw1)[k], 4)
                                         for k in range(KH)]))
    rw2 = np.ascontiguousarray(np.stack([_pack_kt(np.asarray(Rw2)[k], 8)
                                         for k in range(KH)]))
    rw3 = _pack_kt(np.asarray(Rw3, f)[:, :, 0].T, 4)     # [512, 11] -> [128,4,11]

    vecl1 = np.zeros((128, 96, 2), f)
    vecl1[:, 0:8, 0] = _pack_vec(gc1, 8)
    vecl1[:, 0:8, 1] = _pack_vec(bec1, 8)
    for k in range(KH):
        vecl1[:, 8 + 8 * k:16 + 8 * k, 0] = _pack_vec(np.asarray(Rg1)[k], 8)
        vecl1[:, 8 + 8 * k:16 + 8 * k, 1] = _pack_vec(np.asarray(Rbe1)[k], 8)
    vecl2 = np.zeros((128, 60, 2), f)
    vecl2[:, 0:16, 0] = _pack_vec(gc2, 16)
    vecl2[:, 0:16, 1] = _pack_vec(bec2, 16)
    for k in range(KH):
        vecl2[:, 16 + 4 * k:20 + 4 * k, 0] = _pack_vec(np.asarray(Rg2)[k], 4)
        vecl2[:, 16 + 4 * k:20 + 4 * k, 1] = _pack_vec(np.asarray(Rbe2)[k], 4)
    row11 = np.zeros((128, 4, KH), f)
    row11[:, 0, :] = np.asarray(bc3, f)
    row11[:, 1, :] = np.asarray(Rb3, f)[:, 0]
    row11[:, 2, :] = np.arange(KH, dtype=f) - 6.0
    eye11 = np.eye(KH, dtype=f)

    common = dict(wc1=wc1, wc2=wc2, wc3=wc3, rw1=rw1, rw2=rw2, rw3=rw3,
                  vecl1=vecl1, vecl2=vecl2, row11=row11, eye11=eye11)
    in_maps = []
    for c in range(NCORES):
        xc = x[c * BC:(c + 1) * BC]                      # [2048, 512]
        xn = np.ascontiguousarray(xc.reshape(16, 128, 512).transpose(1, 0, 2))
        xt = np.ascontiguousarray(xc.T.reshape(4, 128, 2048).transpose(1, 0, 2))
        k0, k1 = c, c + NCORES if c + NCORES < KH else c
        rw1m = np.ascontiguousarray(np.stack([rw1[k0], rw1[k1]]))
        in_maps.append(dict(common, xn=xn, xt=xt, rw1m=rw1m))

    res = run_bass_kernel_spmd(nc, in_maps, core_ids=list(range(NCORES)),
                               trace=TRACE)
    global LAST_RESULT
    LAST_RESULT = res

    finals, logits, pcts, atpms = [], [], [], []
    for c in range(NCORES):
        r = res.results[c]
        finals.append(r["of"].T.reshape(BC, 1))
        logits.append(r["ol"].transpose(1, 0, 2).reshape(BC, KH))
        pcts.append(r["op"].transpose(1, 0, 2).reshape(BC, KH))
        atpms.append(r["oa"].T.reshape(BC, 1))
    return (np.ascontiguousarray(np.concatenate(finals)),
            np.ascontiguousarray(np.concatenate(logits)),
            np.ascontiguousarray(np.concatenate(pcts)),
            np.ascontiguousarray(np.concatenate(atpms)))
